# revision 1
# baseline (speedup 1.0000x reference)
"""CondConvNeXtBlock Trainium2 kernel.

Data-parallel over batch: 16 batches -> 8 cores x 2 batches. Weights and the
L=64 token bank are replicated. Everything on-chip runs in a channels-on-
partitions, tokens-on-free layout (x arrives as (B, C, T), which is already
the natural rhs layout for out^T = W^T @ x matmuls), so no activation
transposes are needed anywhere:

  phase A (per batch, per 512-token chunk):
    q^T = Wq^T x + bq                         (PE, f32r)
    e^T = blockdiag(k_h0,k_h1)^T q^T  (2 heads packed per 128-row matmul)
    softmax over the 64 keys without max-subtraction (|e| << 1 by
    construction), denominators via an indicator matmul, normalization by
    broadcasting 1/denom with another tiny matmul
    attn_out^T = blockdiag(v)^T attn^T ;  xres = x + Wo^T attn_out + bo
    xres chunks staged to DRAM (SBUF cannot hold the full T=4096 slab).

  phase B (per batch, per chunk, +/-3 halo from DRAM):
    depthwise conv k=7 as 7 accumulating diag-matmuls into PSUM
    LayerNorm over channels (a partition reduction) via ones-vector matmuls
    for sum / sum-of-squares; scale/shift (per-class, host-gathered) folded
    into two broadcast matmuls + scalar_tensor_tensor ops
    MLP: gelu(W1^T ln + b1) streamed per 128-row tile into 4 persistent
    W2 PSUM accumulators; final residual add; DMA out.

All matmul operands are float32r (verified 1.4e-4 rel err, 4x faster than
fp32). A post-Tile pass splits multi-wait instructions (this walrus build
accepts one sync wait per instruction) onto same-engine NoOps.
"""
import numpy as np

import concourse.bass as bass
import concourse.mybir as mybir
import concourse.tile as tile

AF = mybir.ActivationFunctionType
ALU = mybir.AluOpType
F32 = mybir.dt.float32
F32R = mybir.dt.float32r
BF16 = mybir.dt.bfloat16

B, C, T, L, H, I, NE = 16, 512, 4096, 64, 8, 1536, 4
DH = C // H
P = 128
N_CORES = 8
B_LOC = B // N_CORES
EPS = 1e-6
SCALE2 = float((C / H) ** -0.5)  # softmax scale applied twice, folded into k
CHUNK = 512
KC = C // P    # 4
KI = I // P    # 12
NPAIR = H // 2  # 4 head-pairs, each pair = one 128-row block
NCHUNK = T // CHUNK


def split_multiwaits(nc):
    """Walrus here accepts ONE sync-wait command per instruction; Tile may
    attach several. Hoist extras onto same-engine NoOps placed just before
    the offender (same queue => program order preserves semantics)."""
    n_fixed = 0
    for fn in nc.m.functions:
        for bb in fn.blocks:
            insts = bb.instructions
            i = 0
            while i < len(insts):
                inst = insts[i]
                si = inst.sync_info
                if si is not None and len(si.on_wait) > 1:
                    waits = list(si.on_wait)
                    for j, w in enumerate(waits[:-1]):
                        nop = mybir.InstNoOp(name=f"{inst.name}-ws{j}")
                        nop.engine = inst.engine
                        nop.sync_info = mybir.SyncInfo(on_wait=[w], on_update=[])
                        nc.register_instruction(nop)
                        insts.insert(i, nop)
                        i += 1
                    inst.sync_info = mybir.SyncInfo(
                        on_wait=[waits[-1]], on_update=list(si.on_update)
                    )
                    n_fixed += 1
                i += 1
    return n_fixed


def build(nc):
    x_d = nc.dram_tensor("x", (B_LOC, C, T), F32R, kind="ExternalInput")
    wq_d = nc.dram_tensor("wq", (C, C), F32R, kind="ExternalInput")
    bq_d = nc.dram_tensor("bq", (C,), F32, kind="ExternalInput")
    wkv_d = nc.dram_tensor("wkv", (C, 2 * C), F32R, kind="ExternalInput")
    bkv_d = nc.dram_tensor("bkv", (2 * C,), F32, kind="ExternalInput")
    wo_d = nc.dram_tensor("wo", (C, C), F32R, kind="ExternalInput")
    bo_d = nc.dram_tensor("bo", (C,), F32, kind="ExternalInput")
    dwb_d = nc.dram_tensor("dwb", (C,), F32, kind="ExternalInput")
    cdiag_d = nc.dram_tensor("cdiag", (P, KC * 7 * P), F32R, kind="ExternalInput")
    dlt_d = nc.dram_tensor("dlt", (P, NPAIR * 8), F32R, kind="ExternalInput")
    dblt_d = nc.dram_tensor("dblt", (8, NPAIR * P), F32R, kind="ExternalInput")
    ident_d = nc.dram_tensor("ident", (P, P), F32R, kind="ExternalInput")
    mult1_d = nc.dram_tensor("mult1", (P, 1), F32R, kind="ExternalInput")
    ones1_d = nc.dram_tensor("ones1", (1, P), F32R, kind="ExternalInput")
    z3_d = nc.dram_tensor("z3", (C, 3), F32R, kind="ExternalInput")
    w1_d = nc.dram_tensor("w1", (C, I), F32R, kind="ExternalInput")
    b1_d = nc.dram_tensor("b1", (I,), F32, kind="ExternalInput")
    w2_d = nc.dram_tensor("w2", (I, C), F32R, kind="ExternalInput")
    b2_d = nc.dram_tensor("b2", (C,), F32, kind="ExternalInput")
    aux_d = nc.dram_tensor("aux", (L, C), F32R, kind="ExternalInput")
    se_d = nc.dram_tensor("se", (B_LOC, C), F32, kind="ExternalInput")
    sh_d = nc.dram_tensor("sh", (B_LOC, C), F32, kind="ExternalInput")
    nse_d = nc.dram_tensor("nse", (B_LOC, C), F32R, kind="ExternalInput")
    out_d = nc.dram_tensor("out", (B_LOC, C, T), F32, kind="ExternalOutput")

    with tile.TileContext(nc) as tc, \
         nc.allow_low_precision(reason="f32r is ~1.5e-5 rel; whole pipeline validated vs fp64"):
        with tc.tile_pool(name="persist", bufs=1) as pp, \
             tc.tile_pool(name="dram", bufs=1, space="DRAM") as dp:
            xres_d = dp.tile((B_LOC, C, T), F32R)

            # ---- persistent weights / constants ----
            wq_sb = pp.tile((P, KC, C), F32R)
            wo_sb = pp.tile((P, KC, C), F32R)
            w1_sb = pp.tile((P, KC, I), F32R)
            w2_sb = pp.tile((P, KI, C), F32R)

            bq_pp = pp.tile((P, KC), F32)
            bo_pp = pp.tile((P, KC), F32)
            dwb_pp = pp.tile((P, KC), F32)
            b2_pp = pp.tile((P, KC), F32)
            b1_pp = pp.tile((P, KI), F32)
            bkv_pp = pp.tile((P, 2 * KC), F32)
            nc.sync.dma_start(bkv_pp, bkv_d[:].rearrange("(s p) -> p s", p=P))
            se_pp = pp.tile((P, B_LOC * KC), F32)
            sh_pp = pp.tile((P, B_LOC * KC), F32)
            nse_sb = pp.tile((1, B_LOC * KC, P), F32R)
            ident = pp.tile((P, P), F32R)
            nc.sync.dma_start(ident, ident_d[:])
            conv_diag = pp.tile((P, KC * 7, P), F32R)
            mu_lhsT = pp.tile((P, 1), F32R)
            ones_lhsT = pp.tile((1, P), F32R)
            z3_sb = pp.tile((P, KC, 3), F32R)
            eps_pp = pp.tile((1, 1), F32)
            nc.vector.memset(eps_pp, EPS)
            den_lhsT = pp.tile((P, NPAIR, 8), F32R)
            nc.sync.dma_start(den_lhsT, dlt_d[:].rearrange("p (r m) -> p r m", m=8))
            denb_lhsT = pp.tile((8, NPAIR, P), F32R)
            nc.sync.dma_start(denb_lhsT, dblt_d[:].rearrange("p (r m) -> p r m", m=P))

            # ---- KV bank preprocessing (once per core) ----
            zpp_d = nc.dram_tensor("zpp", (P, NPAIR * P), F32R, kind="ExternalInput")
            Bk = pp.tile((P, NPAIR, P), F32R)
            Bv = pp.tile((P, NPAIR, P), F32R)
            nc.sync.dma_start(Bk, zpp_d[:].rearrange("p (r m) -> p r m", m=P))
            with tc.tile_pool(name="pre", bufs=1) as prp, \
                 tc.tile_pool(name="preps", bufs=2, space="PSUM") as pps:
                wkv_sb = prp.tile((P, KC, 2 * C), F32R)
                nc.sync.dma_start(wkv_sb, wkv_d[:].rearrange("(s p) m -> p s m", p=P))
                aux_raw = prp.tile((L, C), F32R)
                nc.sync.dma_start(aux_raw, aux_d[:])
                auxT = prp.tile((P, KC, L), F32R)
                for s in range(KC):
                    ps_at = pps.tile((P, L), F32R, tag="at")
                    nc.tensor.transpose(ps_at, aux_raw[:, s * P : (s + 1) * P], ident[0:L, 0:L])
                    nc.vector.tensor_copy(auxT[:, s, :], ps_at)
                kvT = prp.tile((P, 2 * KC, L), F32R)
                for m in range(2 * KC):
                    ps = pps.tile((P, L), F32, tag="kv")
                    for kt in range(KC):
                        nc.tensor.matmul(
                            ps, wkv_sb[:, kt, m * P : (m + 1) * P], auxT[:, kt, :],
                            start=(kt == 0), stop=(kt == KC - 1),
                        )
                    if m < KC:  # k part: add bias then apply scale^2
                        nc.vector.tensor_scalar(
                            kvT[:, m, :], ps, bkv_pp[:, m : m + 1], SCALE2,
                            ALU.add, ALU.mult,
                        )
                    else:
                        nc.vector.tensor_scalar_add(kvT[:, m, :], ps, bkv_pp[:, m : m + 1])
                bvt = prp.tile((P, NPAIR, P), F32R)
                nc.sync.dma_start(bvt, zpp_d[:].rearrange("p (r m) -> p r m", m=P))
                for pr in range(NPAIR):
                    nc.vector.tensor_copy(Bk[0:DH, pr, 0:DH], kvT[0:DH, pr, :])
                    nc.vector.tensor_copy(Bk[DH:P, pr, DH:P], kvT[DH:P, pr, :])
                    nc.vector.tensor_copy(bvt[0:DH, pr, 0:DH], kvT[0:DH, KC + pr, :])
                    nc.vector.tensor_copy(bvt[DH:P, pr, DH:P], kvT[DH:P, KC + pr, :])
                    pst = pps.tile((P, P), F32R, tag="tr")
                    nc.tensor.transpose(pst, bvt[:, pr, :], ident)
                    nc.vector.tensor_copy(Bv[:, pr, :], pst)

            # ---- main pipeline: phase A for both batches, then phase B ----
            def phase_a(b, pa, pas):
                def qpart(n):
                    t0 = n * CHUNK
                    x_sb = pa.tile((P, KC, CHUNK), F32R, tag="x", name="x_sb")
                    nc.sync.dma_start(
                        x_sb,
                        x_d[b, :, t0 : t0 + CHUNK].rearrange("(s p) t -> p s t", p=P),
                    )
                    q_sb = pa.tile((P, KC, CHUNK), F32R, tag="q", name="q_sb")
                    for mt in range(KC):
                        ps = pas.tile((P, CHUNK), F32, tag="mm", bufs=6, name="ps_q")
                        for kt in range(KC):
                            nc.tensor.matmul(
                                ps, wq_sb[:, kt, mt * P : (mt + 1) * P],
                                x_sb[:, kt, :],
                                start=(kt == 0), stop=(kt == KC - 1),
                            )
                        nc.vector.tensor_scalar_add(
                            q_sb[:, mt, :], ps, bq_pp[:, mt : mt + 1]
                        )
                    return n, x_sb, q_sb

                def attnpart(st):
                    n, x_sb, q_sb = st
                    t0 = n * CHUNK
                    expe = pa.tile((P, NPAIR, CHUNK), F32R, tag="expe", name="expe")
                    ps_den = pas.tile((8, CHUNK), F32, tag="den", bufs=2, name="ps_den")
                    ps_es = []
                    for pr in range(NPAIR):
                        ps_e = pas.tile((P, CHUNK), F32, tag="mm", bufs=6, name="ps_e")
                        nc.tensor.matmul(
                            ps_e, Bk[:, pr, :], q_sb[:, pr, :], start=True, stop=True
                        )
                        ps_es.append(ps_e)
                    for pr in range(NPAIR):
                        nc.scalar.activation(expe[:, pr, :], ps_es[pr], AF.Exp)
                    for pr in range(NPAIR):
                        nc.tensor.matmul(
                            ps_den, den_lhsT[:, pr, :], expe[:, pr, :],
                            start=(pr == 0), stop=(pr == NPAIR - 1),
                        )
                    recip = pa.tile((8, CHUNK), F32R, tag="recip", name="recip")
                    nc.vector.reciprocal(recip, ps_den)
                    att_o = pa.tile((P, KC, CHUNK), F32R, tag="atto", name="att_o")
                    ps_dbs = []
                    for pr in range(NPAIR):
                        ps_db = pas.tile((P, CHUNK), F32, tag="mm", bufs=6, name="ps_db")
                        nc.tensor.matmul(
                            ps_db, denb_lhsT[:, pr, :], recip, start=True, stop=True
                        )
                        ps_dbs.append(ps_db)
                    attns = []
                    for pr in range(NPAIR):
                        attn = pa.tile((P, CHUNK), F32R, tag="attn", bufs=4, name="attn")
                        nc.vector.tensor_tensor(attn, expe[:, pr, :], ps_dbs[pr], ALU.mult)
                        attns.append(attn)
                    ps_avs = []
                    for pr in range(NPAIR):
                        ps_av = pas.tile((P, CHUNK), F32, tag="mm", bufs=6, name="ps_av")
                        nc.tensor.matmul(ps_av, Bv[:, pr, :], attns[pr], start=True, stop=True)
                        ps_avs.append(ps_av)
                    for pr in range(NPAIR):
                        nc.scalar.activation(att_o[:, pr, :], ps_avs[pr], AF.Copy)
                    xres = pa.tile((P, KC, CHUNK), F32R, tag="xres", name="xres")
                    for mt in range(KC):
                        ps = pas.tile((P, CHUNK), F32, tag="mm", bufs=6, name="ps_wo")
                        for kt in range(KC):
                            nc.tensor.matmul(
                                ps, wo_sb[:, kt, mt * P : (mt + 1) * P],
                                att_o[:, kt, :],
                                start=(kt == 0), stop=(kt == KC - 1),
                            )
                        nc.vector.scalar_tensor_tensor(
                            xres[:, mt, :], ps, bo_pp[:, mt : mt + 1],
                            x_sb[:, mt, :], ALU.add, ALU.add,
                        )
                    nc.sync.dma_start(
                        xres_d[b, :, t0 : t0 + CHUNK].rearrange("(s p) t -> p s t", p=P),
                        xres,
                    )

                for n in range(NCHUNK):
                    attnpart(qpart(n))

            def phase_b(b, pb, pbs):
                def convstats(n):
                    t0 = n * CHUNK
                    xr = pb.tile((P, KC, CHUNK + 6), F32R, tag="xr", bufs=3, name="xr")
                    if n == 0:
                        nc.vector.tensor_copy(xr[:, :, 0:3], z3_sb)
                        nc.sync.dma_start(
                            xr[:, :, 3 : CHUNK + 6],
                            xres_d[b, :, 0 : CHUNK + 3].rearrange("(s p) t -> p s t", p=P),
                        )
                    elif n == NCHUNK - 1:
                        nc.vector.tensor_copy(xr[:, :, CHUNK + 3 : CHUNK + 6], z3_sb)
                        nc.sync.dma_start(
                            xr[:, :, 0 : CHUNK + 3],
                            xres_d[b, :, t0 - 3 : t0 + CHUNK].rearrange("(s p) t -> p s t", p=P),
                        )
                    else:
                        nc.sync.dma_start(
                            xr,
                            xres_d[b, :, t0 - 3 : t0 + CHUNK + 3].rearrange("(s p) t -> p s t", p=P),
                        )
                    y = pb.tile((P, KC, CHUNK), F32R, tag="y", name="y")
                    ysq = pb.tile((P, KC, CHUNK), F32R, tag="ysq", bufs=1, name="ysq")
                    ps_mu = pbs.tile((1, CHUNK), F32, tag="small", bufs=2, name="ps_mu")
                    ps_sq = pbs.tile((1, CHUNK), F32, tag="small", bufs=2, name="ps_sq")
                    for pt in range(KC):
                        ps = pbs.tile((P, CHUNK), F32, tag="cv", bufs=2, name="ps_cv")
                        for k in range(7):
                            nc.tensor.matmul(
                                ps, conv_diag[:, pt * 7 + k, :],
                                xr[:, pt, k : k + CHUNK],
                                start=(k == 0), stop=(k == 6),
                            )
                        nc.vector.tensor_scalar_add(y[:, pt, :], ps, dwb_pp[:, pt : pt + 1])
                        nc.scalar.activation(
                            ysq[:, pt, :], ps, AF.Square, bias=dwb_pp[:, pt : pt + 1]
                        )
                        nc.tensor.matmul(
                            ps_mu, mu_lhsT, y[:, pt, :],
                            start=(pt == 0), stop=(pt == KC - 1), skip_group_check=True,
                        )
                        nc.tensor.matmul(
                            ps_sq, mu_lhsT, ysq[:, pt, :],
                            start=(pt == 0), stop=(pt == KC - 1), skip_group_check=True,
                        )
                    # evacuate stats so the small-psum slots free quickly
                    su = pb.tile((1, CHUNK), F32, tag="su", name="su")
                    ssq = pb.tile((1, CHUNK), F32, tag="ssq", name="ssq")
                    nc.scalar.activation(su, ps_mu, AF.Copy)
                    nc.scalar.activation(ssq, ps_sq, AF.Copy)
                    return n, xr, y, su, ssq

                def lnmlp(st):
                    n, xr, y, su, ssq = st
                    t0 = n * CHUNK
                    rowA = pb.tile((1, CHUNK), F32R, tag="rowA", name="rowA")
                    rowB = pb.tile((1, CHUNK), F32R, tag="rowB", name="rowB")
                    musq = pb.tile((1, CHUNK), F32, tag="musq", name="musq")
                    nc.vector.tensor_tensor(musq, su, su, ALU.mult)
                    varr = pb.tile((1, CHUNK), F32, tag="varr", name="varr")
                    nc.vector.tensor_sub(varr, ssq, musq)
                    sqv = pb.tile((1, CHUNK), F32, tag="sqv", name="sqv")
                    nc.scalar.activation(sqv, varr, AF.Sqrt, bias=eps_pp[0:1, 0:1])
                    nc.vector.reciprocal(rowA, sqv)  # A = rstd
                    nc.vector.tensor_tensor(rowB, su, rowA, ALU.mult)  # mu * A
                    ps_ab = pbs.tile((P, CHUNK), F32, tag="small", bufs=2, name="ps_ab")
                    nc.tensor.matmul(ps_ab, ones_lhsT, rowA, start=True, stop=True)
                    ln = pb.tile((P, KC, CHUNK), F32R, tag="ln", bufs=1, name="ln")
                    for pt in range(KC):
                        col = b * KC + pt
                        ps_b = pbs.tile((P, CHUNK), F32, tag="small", bufs=2, name="ps_b")
                        nc.tensor.matmul(
                            ps_b, nse_sb[:, col, :], rowB, start=True, stop=True
                        )
                        lntmp = pb.tile((P, CHUNK), F32, tag="lntmp", bufs=1, name="lntmp")
                        nc.vector.scalar_tensor_tensor(
                            lntmp, y[:, pt, :], se_pp[:, col : col + 1], ps_ab,
                            ALU.mult, ALU.mult,
                        )
                        nc.vector.scalar_tensor_tensor(
                            ln[:, pt, :], lntmp, sh_pp[:, col : col + 1], ps_b,
                            ALU.add, ALU.add,
                        )
                    hall = pb.tile((P, KI, CHUNK), F32R, tag="hall", bufs=1, name="hall")
                    for i_ in range(KI):
                        ps_h = pbs.tile((P, CHUNK), F32, tag="w1", bufs=2, name="ps_h")
                        for kt in range(KC):
                            nc.tensor.matmul(
                                ps_h, w1_sb[:, kt, i_ * P : (i_ + 1) * P],
                                ln[:, kt, :],
                                start=(kt == 0), stop=(kt == KC - 1),
                            )
                        nc.scalar.activation(
                            hall[:, i_, :], ps_h, AF.Gelu, bias=b1_pp[:, i_ : i_ + 1]
                        )
                    outt = pb.tile((P, KC, CHUNK), F32, tag="outt", bufs=1, name="outt")
                    for mt in range(KC):
                        ps_o = pbs.tile((P, CHUNK), F32, tag="w2", bufs=2, name="ps_o")
                        for i_ in range(KI):
                            nc.tensor.matmul(
                                ps_o, w2_sb[:, i_, mt * P : (mt + 1) * P],
                                hall[:, i_, :],
                                start=(i_ == 0), stop=(i_ == KI - 1),
                            )
                        nc.vector.scalar_tensor_tensor(
                            outt[:, mt, :], ps_o, b2_pp[:, mt : mt + 1],
                            xr[:, mt, 3 : 3 + CHUNK], ALU.add, ALU.add,
                        )
                    nc.sync.dma_start(
                        out_d[b, :, t0 : t0 + CHUNK].rearrange("(s p) t -> p s t", p=P),
                        outt,
                    )

                for n in range(NCHUNK):
                    lnmlp(convstats(n))

            # attention weights stream while the KV bank computes
            nc.sync.dma_start(wq_sb, wq_d[:].rearrange("(s p) m -> p s m", p=P))
            nc.sync.dma_start(wo_sb, wo_d[:].rearrange("(s p) m -> p s m", p=P))
            nc.sync.dma_start(bq_pp, bq_d[:].rearrange("(s p) -> p s", p=P))
            nc.sync.dma_start(bo_pp, bo_d[:].rearrange("(s p) -> p s", p=P))
            with tc.tile_pool(name="pa", bufs=2) as pa,                  tc.tile_pool(name="pas", bufs=1, space="PSUM") as pas:
                for b in range(B_LOC):
                    phase_a(b, pa, pas)
            # phase-B weights stream in while phase A computes
            nc.sync.dma_start(w1_sb, w1_d[:].rearrange("(s p) m -> p s m", p=P))
            nc.sync.dma_start(w2_sb, w2_d[:].rearrange("(s p) m -> p s m", p=P))
            nc.sync.dma_start(dwb_pp, dwb_d[:].rearrange("(s p) -> p s", p=P))
            nc.sync.dma_start(b2_pp, b2_d[:].rearrange("(s p) -> p s", p=P))
            nc.sync.dma_start(b1_pp, b1_d[:].rearrange("(s p) -> p s", p=P))
            nc.sync.dma_start(se_pp, se_d[:].rearrange("b (s p) -> p (b s)", p=P))
            nc.sync.dma_start(sh_pp, sh_d[:].rearrange("b (s p) -> p (b s)", p=P))
            nc.sync.dma_start(nse_sb, nse_d[:].rearrange("b (s m) -> (b s) m", m=P)[None])
            nc.sync.dma_start(
                conv_diag, cdiag_d[:].rearrange("p (q m) -> p q m", m=P)
            )
            nc.sync.dma_start(mu_lhsT, mult1_d[:])
            nc.sync.dma_start(ones_lhsT, ones1_d[:])
            nc.sync.dma_start(z3_sb, z3_d[:].rearrange("(s p) k -> p s k", p=P))
            with tc.tile_pool(name="pb", bufs=2) as pb,                  tc.tile_pool(name="pbs", bufs=1, space="PSUM") as pbs:
                for b in range(B_LOC):
                    phase_b(b, pb, pbs)
    return nc


_CACHE = {}


def _get_nc():
    if "nc" not in _CACHE:
        nc = bass.Bass()
        build(nc)
        split_multiwaits(nc)
        _CACHE["nc"] = nc
    return _CACHE["nc"]


def kernel(**inputs):
    from concourse.bass_utils import run_bass_kernel_spmd

    f = lambda k: np.ascontiguousarray(np.asarray(inputs[k], dtype=np.float32))
    x = f("x")
    ids = np.asarray(inputs["cond_embedding_id"]).astype(np.int64)
    se_all = f("scale_emb")[ids]   # (B, C)
    sh_all = f("shift_emb")[ids]
    wq, bq = f("Wq"), f("bq")
    wkv, bkv = f("Wkv"), f("bkv")
    wo, bo = f("Wo"), f("bo")
    dww = f("dw_w").reshape(C, 7)
    dwb = f("dw_b")
    ident_h = np.eye(P, dtype=np.float32)
    # conv diag stack: cdiag[c, k*P + m] = dww[c, k] if (c % P) == m
    # cdiag[p, s, k, m] = dww[s*P + p, k] * (m == p)
    cdiag = np.zeros((P, KC, 7, P), dtype=np.float32)
    pp_ = np.arange(P)
    for s in range(KC):
        for k in range(7):
            cdiag[pp_, s, k, pp_] = dww[s * P + pp_, k]
    cdiag = cdiag.reshape(P, KC * 7 * P)
    # softmax denominator indicators
    dlt = np.zeros((P, NPAIR, 8), dtype=np.float32)
    dblt = np.zeros((8, NPAIR, P), dtype=np.float32)
    for pr in range(NPAIR):
        dlt[0:DH, pr, 2 * pr] = 1.0
        dlt[DH:P, pr, 2 * pr + 1] = 1.0
        dblt[2 * pr, pr, 0:DH] = 1.0
        dblt[2 * pr + 1, pr, DH:P] = 1.0
    dlt = dlt.reshape(P, NPAIR * 8)
    dblt = dblt.reshape(8, NPAIR * P)
    mult1 = np.full((P, 1), 1.0 / C, dtype=np.float32)
    ones1 = np.ones((1, P), dtype=np.float32)
    z3 = np.zeros((C, 3), dtype=np.float32)
    zpp = np.zeros((P, NPAIR * P), dtype=np.float32)
    w1, b1 = f("W1"), f("b1")
    w2, b2 = f("W2"), f("b2")
    aux = f("aux")

    in_maps = []
    for c in range(N_CORES):
        sl = slice(c * B_LOC, (c + 1) * B_LOC)
        in_maps.append({
            "x": np.ascontiguousarray(x[sl]),
            "wq": wq, "bq": bq, "wkv": wkv, "bkv": bkv, "wo": wo, "bo": bo,
            "dwb": dwb, "w1": w1, "b1": b1, "w2": w2, "b2": b2,
            "cdiag": cdiag, "dlt": dlt, "dblt": dblt, "ident": ident_h,
            "mult1": mult1, "ones1": ones1, "z3": z3, "zpp": zpp,
            "aux": aux,
            "se": np.ascontiguousarray(se_all[sl]),
            "sh": np.ascontiguousarray(sh_all[sl]),
            "nse": np.ascontiguousarray(-se_all[sl]),
        })

    nc = _get_nc()
    _CACHE["in_maps"] = in_maps
    res = run_bass_kernel_spmd(nc, in_maps, core_ids=list(range(N_CORES)))
    return np.concatenate([r["out"] for r in res.results], axis=0)


if __name__ == "__main__":
    nc = bass.Bass()
    build(nc)
    n = split_multiwaits(nc)
    print("built; multiwait splits:", n)



# revision 13
# speedup vs baseline: 1.2320x; 1.2320x over previous
"""CondConvNeXtBlock Trainium2 kernel, v2 (fp8-DoubleRow single-pass).

Data-parallel over batch: 16 batches -> 8 cores x 2 batches. Weights and the
L=64 token bank are replicated. Channels-on-partitions, tokens-on-free layout
throughout; all heavy matmuls run as fp8e4 DoubleRow (2 contraction subtiles
per instruction at 0.5 cycles/row = 4x f32r).

Key structural moves vs v1:
  * Q-projection is folded into the key bank on the host: e = K''^T x with
    K'' = Wq @ k (per head), so the Q matmul, its bias add and its PSUM
    evacuation disappear. The bias terms (bq, and the +b2+bo folded into x)
    become per-key constants added inside the softmax Exp activation.
  * Softmax normalization is applied post-attention: avu = V^T expe
    (unnormalized) and att8 = avu * bcast(256/den) in one tensor_tensor that
    also quantizes to fp8 for the Wo DoubleRow matmul.
  * Single pass: xres chunks stay in SBUF (f32r ring + fp8 conv slab written
    at +0/+1 shifts so conv taps pair into DoubleRow subtiles). No DRAM
    round trip between attention and conv.
  * AdaLayerNorm: scale_emb folds into W1 (host, per batch class), shift_emb
    into b1, bo/b2 into x / dw_b. rstd is broadcast once per chunk via a
    ones-matmul; y8ln = y8 * bcast(rstd) quantizes straight to fp8. The
    -mu*rstd correction enters the W1 PSUM group as a 1-partition DoubleRow
    row, so no second LN elementwise op exists.
  * Elementwise ops are spread across Act/DVE/Pool(gpsimd) to keep every
    engine below the PE roofline.

Scales (all powers of two, folded into weights on the host or into
activation `scale` arguments): K''*1024, Wo*64, att8=256*av, conv w*64,
y8=16*y, W1*64*se, W2*64, gelu descale 1/1024, output descale 1/64.
"""
import numpy as np
import ml_dtypes

import concourse.bass as bass
import concourse.mybir as mybir
import concourse.tile as tile

AF = mybir.ActivationFunctionType
ALU = mybir.AluOpType
F32 = mybir.dt.float32
F32R = mybir.dt.float32r
FP8 = mybir.dt.float8e4
DR = mybir.MatmulPerfMode.DoubleRow
E4 = ml_dtypes.float8_e4m3

B, C, T, L, H, I, NE = 16, 512, 4096, 64, 8, 1536, 4
DH = C // H
P = 128
N_CORES = 8
B_LOC = B // N_CORES
EPS = 1e-6
SCALE2 = float((C / H) ** -0.5)
CHUNK = 512
KC = C // P     # 4
KI = I // P     # 12
NPAIR = H // 2  # 4
NCHUNK = T // CHUNK
TS = T + 8      # fp8 conv slab width (4 pad left, 4 right)

SK = 1024.0     # K'' scale
SWO = 64.0      # Wo scale
SAO = 256.0     # att8 = 256 * av (from den-side 1/256)
SCV = 64.0      # conv weight scale
SY = 16.0       # y8 = 16 * y
SW1 = 64.0
SW2 = 64.0


def split_multiwaits(nc):
    """Walrus here accepts ONE sync-wait command per instruction; Tile may
    attach several. Hoist extras onto same-engine NoOps placed just before
    the offender (same queue => program order preserves semantics)."""
    n_fixed = 0
    for fn in nc.m.functions:
        for bb in fn.blocks:
            insts = bb.instructions
            i = 0
            while i < len(insts):
                inst = insts[i]
                si = inst.sync_info
                if si is not None and len(si.on_wait) > 1:
                    waits = list(si.on_wait)
                    for j, w in enumerate(waits[:-1]):
                        nop = mybir.InstNoOp(name=f"{inst.name}-ws{j}")
                        nop.engine = inst.engine
                        nop.sync_info = mybir.SyncInfo(on_wait=[w], on_update=[])
                        nc.register_instruction(nop)
                        insts.insert(i, nop)
                        i += 1
                    inst.sync_info = mybir.SyncInfo(
                        on_wait=[waits[-1]], on_update=list(si.on_update)
                    )
                    n_fixed += 1
                i += 1
    return n_fixed


def build(nc):
    x_d = nc.dram_tensor("x", (B_LOC, C, T), F32R, kind="ExternalInput")
    x8_d = nc.dram_tensor("x8", (B_LOC, C, T), FP8, kind="ExternalInput")
    k8_d = nc.dram_tensor("k8", (P, KC * NPAIR * P), FP8, kind="ExternalInput")
    cj_d = nc.dram_tensor("cj", (P, NPAIR), F32, kind="ExternalInput")
    bv_d = nc.dram_tensor("bv", (P, NPAIR * P), FP8, kind="ExternalInput")
    dlt_d = nc.dram_tensor("dlt", (P, NPAIR * 8), F32R, kind="ExternalInput")
    dblt_d = nc.dram_tensor("dblt", (8, NPAIR * P), F32R, kind="ExternalInput")
    wo8_d = nc.dram_tensor("wo8", (P, KC * C), FP8, kind="ExternalInput")
    cd8_d = nc.dram_tensor("cd8", (P, KC * 4 * 2 * P), FP8, kind="ExternalInput")
    m18_d = nc.dram_tensor("m18", (P, 2 * 32), FP8, kind="ExternalInput")
    ones4_d = nc.dram_tensor("ones4", (1, P), F32R, kind="ExternalInput")
    w18_d = nc.dram_tensor("w18", (P, B_LOC * KC * I), FP8, kind="ExternalInput")
    w1s8_d = nc.dram_tensor("w1s8", (1, B_LOC * 2 * KI * P), FP8, kind="ExternalInput")
    b1p_d = nc.dram_tensor("b1p", (P, B_LOC * KI), F32, kind="ExternalInput")
    w28_d = nc.dram_tensor("w28", (P, KI * C), FP8, kind="ExternalInput")
    dwb16_d = nc.dram_tensor("dwb16", (P, KC), F32, kind="ExternalInput")
    out_d = nc.dram_tensor("out", (B_LOC, C, T), F32, kind="ExternalOutput")

    with tile.TileContext(nc) as tc, \
         nc.allow_low_precision(reason="fp8 matmuls validated vs fp32 reference"):
        with tc.tile_pool(name="persist", bufs=1) as pp:
            k8 = pp.tile((P, KC, NPAIR * P), FP8)
            nc.sync.dma_start(k8, k8_d[:].rearrange("p (s m) -> p s m", m=NPAIR * P))
            cj = pp.tile((P, NPAIR), F32)
            nc.sync.dma_start(cj, cj_d[:])
            bv = pp.tile((P, NPAIR, P), FP8)
            nc.sync.dma_start(bv, bv_d[:].rearrange("p (r m) -> p r m", m=P))
            dlt = pp.tile((P, NPAIR, 8), F32R)
            nc.sync.dma_start(dlt, dlt_d[:].rearrange("p (r m) -> p r m", m=8))
            dblt = pp.tile((8, NPAIR, P), F32R)
            nc.sync.dma_start(dblt, dblt_d[:].rearrange("p (r m) -> p r m", m=P))
            wo8 = pp.tile((P, KC, C), FP8)
            nc.sync.dma_start(wo8, wo8_d[:].rearrange("p (s m) -> p s m", m=C))
            cd8 = pp.tile((P, KC, 4, 2, P), FP8)
            nc.sync.dma_start(
                cd8, cd8_d[:].rearrange("p (s g r m) -> p s g r m", s=KC, g=4, r=2)
            )
            m18 = pp.tile((P, 2, 32), FP8)
            nc.sync.dma_start(m18, m18_d[:].rearrange("p (r m) -> p r m", m=32))
            ones4 = pp.tile((1, P), F32R)
            nc.sync.dma_start(ones4, ones4_d[:])
            w18 = pp.tile((P, B_LOC, KC, I), FP8)
            nc.sync.dma_start(
                w18, w18_d[:].rearrange("p (b s m) -> p b s m", b=B_LOC, s=KC)
            )
            w1s8 = pp.tile((1, B_LOC, 2, KI * P), FP8)
            nc.sync.dma_start(
                w1s8, w1s8_d[:].rearrange("p (b r m) -> p b r m", b=B_LOC, r=2)
            )
            b1p = pp.tile((P, B_LOC, KI), F32)
            nc.sync.dma_start(b1p, b1p_d[:].rearrange("p (b m) -> p b m", b=B_LOC))
            w28 = pp.tile((P, KI, C), FP8)
            nc.sync.dma_start(w28, w28_d[:].rearrange("p (s m) -> p s m", m=C))
            dwb16 = pp.tile((P, KC), F32)
            nc.sync.dma_start(dwb16, dwb16_d[:])
            eps_pp = pp.tile((1, 1), F32)
            nc.vector.memset(eps_pp, EPS)
            # fp8 conv slab: [:, 0] holds xres8, [:, 1] the same shifted left
            # by one column, so taps (2g, 2g+1) form one DoubleRow pair.
            slab = pp.tile((P, 2, KC, TS), FP8)

            with tc.tile_pool(name="pa", bufs=2) as pa, \
                 tc.tile_pool(name="ps", bufs=1, space="PSUM") as pas:

                MMB = 5

                def phase_a1(b, n):
                    """Attention up to the fp8 normalized context att8."""
                    t0 = n * CHUNK
                    x8t = pa.tile((P, KC, CHUNK), FP8, tag="x8t", name="x8t")
                    nc.sync.dma_start(
                        x8t,
                        x8_d[b, :, t0 : t0 + CHUNK].rearrange("(s p) t -> p s t", p=P),
                    )
                    xst = pa.tile((P, KC, CHUNK), F32R, tag="xst", name="xst")
                    nc.sync.dma_start(
                        xst,
                        x_d[b, :, t0 : t0 + CHUNK].rearrange("(s p) t -> p s t", p=P),
                    )
                    ps_es = []
                    for pr in range(NPAIR):
                        ps_e = pas.tile((P, CHUNK), F32, tag="mm", bufs=MMB, name="ps_e")
                        for g in range(2):
                            nc.tensor.matmul(
                                ps_e, k8[:, 2 * g : 2 * g + 2, pr * P : (pr + 1) * P],
                                x8t[:, 2 * g : 2 * g + 2, :],
                                start=(g == 0), stop=(g == 1), perf_mode=DR,
                            )
                        ps_es.append(ps_e)
                    expe = pa.tile((P, NPAIR, CHUNK), F32R, tag="expe", name="expe")
                    for pr in range(NPAIR):
                        nc.scalar.activation(
                            expe[:, pr, :], ps_es[pr], AF.Exp,
                            bias=cj[:, pr : pr + 1], scale=1.0 / SK,
                        )
                    ps_den = pas.tile((8, CHUNK), F32, tag="den", bufs=1, name="ps_den")
                    for pr in range(NPAIR):
                        nc.tensor.matmul(
                            ps_den, dlt[:, pr, :], expe[:, pr, :],
                            start=(pr == 0), stop=(pr == NPAIR - 1),
                        )
                    recip = pa.tile((8, CHUNK), F32R, tag="recip", name="recip")
                    nc.vector.reciprocal(recip, ps_den)
                    attn8 = pa.tile((P, NPAIR, CHUNK), FP8, tag="attn8", name="attn8")
                    att8 = pa.tile((P, KC, CHUNK), FP8, tag="att8", name="att8")
                    for pr in range(NPAIR):
                        ps_db = pas.tile((P, CHUNK), F32, tag="mm", bufs=MMB, name="ps_db")
                        nc.tensor.matmul(ps_db, dblt[:, pr, :], recip, start=True, stop=True)
                        nc.vector.tensor_tensor(
                            attn8[:, pr, :], expe[:, pr, :], ps_db, ALU.mult
                        )
                        ps_av = pas.tile((P, CHUNK), F32, tag="mm", bufs=MMB, name="ps_av")
                        nc.tensor.matmul(ps_av, bv[:, pr, :], attn8[:, pr, :], start=True, stop=True)
                        nc.scalar.activation(att8[:, pr, :], ps_av, AF.Copy)
                    return {"att8": att8, "xst": xst}

                def phase_a2(b, n, st):
                    """Wo projection, residual, and the two shifted fp8 slab
                    writes feeding the conv DoubleRow pairs."""
                    t0 = n * CHUNK
                    att8, xst = st["att8"], st["xst"]
                    xres = pa.tile((P, KC, CHUNK), F32R, tag="xres", bufs=3, name="xres")
                    for mt in range(KC):
                        ps_wo = pas.tile((P, CHUNK), F32, tag="mm", bufs=MMB, name="ps_wo")
                        for g in range(2):
                            nc.tensor.matmul(
                                ps_wo, wo8[:, 2 * g : 2 * g + 2, mt * P : (mt + 1) * P],
                                att8[:, 2 * g : 2 * g + 2, :],
                                start=(g == 0), stop=(g == 1), perf_mode=DR,
                            )
                        nc.vector.scalar_tensor_tensor(
                            xres[:, mt, :], ps_wo, 1.0 / (SWO * SAO),
                            xst[:, mt, :], ALU.mult, ALU.add,
                        )
                    nc.vector.tensor_copy(slab[:, 0, :, t0 + 4 : t0 + 516], xres)
                    nc.vector.tensor_copy(slab[:, 1, :, t0 + 3 : t0 + 515], xres)
                    st["xres"] = xres

                def phase_b(b, m, xres_m):
                    t0 = m * CHUNK
                    y8 = pa.tile((P, KC, CHUNK), FP8, tag="y8", name="y8")
                    for pt in range(KC):
                        ps_cv = pas.tile((P, CHUNK), F32, tag="mm", bufs=MMB, name="ps_cv")
                        for g in range(4):
                            nc.tensor.matmul(
                                ps_cv, cd8[:, pt, g],
                                slab[:, :, pt, t0 + 2 * g + 1 : t0 + 2 * g + 513],
                                start=(g == 0), stop=(g == 3), perf_mode=DR,
                            )
                        nc.scalar.activation(
                            y8[:, pt, :], ps_cv, AF.Identity,
                            bias=dwb16[:, pt : pt + 1], scale=SY / SCV,
                        )
                    ysq8 = pa.tile((P, KC, CHUNK), FP8, tag="ysq", name="ysq8")
                    for pt in range(KC):
                        nc.gpsimd.tensor_tensor(
                            ysq8[:, pt, :], y8[:, pt, :], y8[:, pt, :], ALU.mult
                        )
                    ps_ms = pas.tile((32, 2 * CHUNK), F32, tag="sm", bufs=1, name="ps_ms")
                    for g in range(2):
                        nc.tensor.matmul(
                            ps_ms[:, 0:CHUNK], m18, y8[:, 2 * g : 2 * g + 2, :],
                            start=(g == 0), stop=(g == 1), perf_mode=DR,
                            skip_group_check=True,
                        )
                    for g in range(2):
                        nc.tensor.matmul(
                            ps_ms[:, CHUNK : 2 * CHUNK], m18, ysq8[:, 2 * g : 2 * g + 2, :],
                            start=(g == 0), stop=(g == 1), perf_mode=DR,
                            skip_group_check=True,
                        )
                    su = pa.tile((1, CHUNK), F32, tag="su", name="su")
                    nc.scalar.activation(su, ps_ms[0:1, 0:CHUNK], AF.Copy)
                    ssq = pa.tile((1, CHUNK), F32, tag="ssq", name="ssq")
                    nc.vector.tensor_copy(ssq, ps_ms[0:1, CHUNK : 2 * CHUNK])
                    musq = pa.tile((1, CHUNK), F32, tag="musq", name="musq")
                    nc.gpsimd.tensor_tensor(musq, su, su, ALU.mult)
                    varr = pa.tile((1, CHUNK), F32, tag="varr", name="varr")
                    nc.gpsimd.tensor_sub(varr, ssq, musq)
                    sqv = pa.tile((1, CHUNK), F32, tag="sqv", name="sqv")
                    nc.scalar.activation(sqv, varr, AF.Sqrt, bias=eps_pp[0:1, 0:1], scale=1.0 / (SY * SY))
                    rowA = pa.tile((1, CHUNK), F32R, tag="rowA", name="rowA")
                    nc.vector.reciprocal(rowA, sqv)
                    brhs = pa.tile((1, 2, CHUNK), FP8, tag="brhs", name="brhs")
                    nc.gpsimd.memset(brhs[:, 1, :], 0.0)
                    nc.gpsimd.tensor_tensor(brhs[:, 0, :], su, rowA, ALU.mult)
                    ps_ab = pas.tile((P, CHUNK), F32, tag="mm", bufs=MMB, name="ps_ab")
                    nc.tensor.matmul(ps_ab, ones4, rowA, start=True, stop=True)
                    y8ln = pa.tile((P, KC, CHUNK), FP8, tag="y8ln", name="y8ln")
                    for pt in range(KC):
                        nc.vector.tensor_tensor(y8ln[:, pt, :], y8[:, pt, :], ps_ab, ALU.mult)
                    hall8 = pa.tile((P, KI, CHUNK), FP8, tag="hall", name="hall8")
                    for i_ in range(KI):
                        ps_h = pas.tile((P, CHUNK), F32, tag="mm", bufs=MMB, name="ps_h")
                        for g in range(2):
                            nc.tensor.matmul(
                                ps_h, w18[:, b, 2 * g : 2 * g + 2, i_ * P : (i_ + 1) * P],
                                y8ln[:, 2 * g : 2 * g + 2, :],
                                start=(g == 0), stop=False, perf_mode=DR,
                            )
                        nc.tensor.matmul(
                            ps_h, w1s8[:, b, :, i_ * P : (i_ + 1) * P], brhs,
                            start=False, stop=True, perf_mode=DR,
                        )
                        nc.scalar.activation(
                            hall8[:, i_, :], ps_h, AF.Gelu,
                            bias=b1p[:, b, i_ : i_ + 1], scale=1.0 / (SY * SW1),
                        )
                    outt = pa.tile((P, KC, CHUNK), F32, tag="outt", name="outt")
                    for mt in range(KC):
                        ps_o = pas.tile((P, CHUNK), F32, tag="mm", bufs=MMB, name="ps_o")
                        for j in range(KI // 2):
                            nc.tensor.matmul(
                                ps_o, w28[:, 2 * j : 2 * j + 2, mt * P : (mt + 1) * P],
                                hall8[:, 2 * j : 2 * j + 2, :],
                                start=(j == 0), stop=(j == KI // 2 - 1), perf_mode=DR,
                            )
                        nc.vector.scalar_tensor_tensor(
                            outt[:, mt, :], ps_o, 1.0 / SW2,
                            xres_m[:, mt, :], ALU.mult, ALU.add,
                        )
                    nc.sync.dma_start(
                        out_d[b, :, t0 : t0 + CHUNK].rearrange("(s p) t -> p s t", p=P),
                        outt,
                    )

                for b in range(B_LOC):
                    nc.vector.memset(slab[:, :, :, 0:4], 0.0)
                    nc.vector.memset(slab[:, :, :, T + 3 : TS], 0.0)
                    state = {}
                    for it in range(NCHUNK + 2):
                        if it < NCHUNK:
                            state[it] = phase_a1(b, it)
                        if 1 <= it <= NCHUNK:
                            phase_a2(b, it - 1, state[it - 1])
                        if it >= 2:
                            phase_b(b, it - 2, state.pop(it - 2)["xres"])
    return nc


_CACHE = {}


def _get_nc():
    if "nc" not in _CACHE:
        nc = bass.Bass()
        build(nc)
        split_multiwaits(nc)
        _CACHE["nc"] = nc
    return _CACHE["nc"]


def _prep(inputs):
    f = lambda k: np.asarray(inputs[k], dtype=np.float32)
    x = f("x")
    ids = np.asarray(inputs["cond_embedding_id"]).astype(np.int64)
    se_all = f("scale_emb")[ids]   # (B, C)
    sh_all = f("shift_emb")[ids]
    wq, bq = f("Wq"), f("bq")
    wkv, bkv = f("Wkv"), f("bkv")
    wo, bo = f("Wo"), f("bo")
    dww = f("dw_w").reshape(C, 7)
    dwb = f("dw_b")
    w1, b1 = f("W1"), f("b1")
    w2, b2 = f("W2"), f("b2")
    aux = f("aux")

    kv = aux @ wkv + bkv                      # (L, 2C)
    k_eff = kv[:, :C] * SCALE2                # (L, C)
    v = kv[:, C:]                             # (L, C)
    b2p = b2 + bo                             # folded into x on the host

    # K''[c, pr, jj] = sum_dh Wq[c, h*64+dh] * k_eff[j, h*64+dh]
    kh = k_eff.reshape(L, H, DH)              # (L, H, DH)
    wqh = wq.reshape(C, H, DH)                # (C, H, DH)
    kpp = np.einsum("chd,lhd->clh", wqh, kh)  # (C, L, H)
    k8 = np.zeros((P, KC, NPAIR, P), dtype=np.float32)
    cjv = np.zeros((P, NPAIR), dtype=np.float32)
    bqh = bq.reshape(H, DH)
    for pr in range(NPAIR):
        for half in range(2):
            h = 2 * pr + half
            jj = slice(half * DH, (half + 1) * DH)
            blk = kpp[:, :, h]                # (C, L)
            k8[:, :, pr, jj] = (SK * blk).reshape(KC, P, L).transpose(1, 0, 2)
            cjv[jj, pr] = kh[:, h, :] @ bqh[h] - blk.T @ b2p
    k8 = np.clip(k8, -240, 240).astype(E4).reshape(P, KC * NPAIR * P)

    bvv = np.zeros((P, NPAIR, P), dtype=np.float32)
    for pr in range(NPAIR):
        for half in range(2):
            h = 2 * pr + half
            s = slice(half * DH, (half + 1) * DH)
            bvv[s, pr, s] = v[:, h * DH : (h + 1) * DH]
    bvv = np.clip(16.0 * bvv, -240, 240).astype(E4).reshape(P, NPAIR * P)

    dlt = np.zeros((P, NPAIR, 8), dtype=np.float32)
    dblt = np.zeros((8, NPAIR, P), dtype=np.float32)
    for pr in range(NPAIR):
        dlt[0:DH, pr, 2 * pr] = 1.0 / 16.0
        dlt[DH:P, pr, 2 * pr + 1] = 1.0 / 16.0
        dblt[2 * pr, pr, 0:DH] = 1.0
        dblt[2 * pr + 1, pr, DH:P] = 1.0
    dlt = dlt.reshape(P, NPAIR * 8)
    dblt = dblt.reshape(8, NPAIR * P)

    wo8 = np.clip(
        SWO * wo.reshape(KC, P, C).transpose(1, 0, 2), -240, 240
    ).astype(E4).reshape(P, KC * C)

    cd8 = np.zeros((P, KC, 4, 2, P), dtype=np.float32)
    pp_ = np.arange(P)
    for s in range(KC):
        for kk in range(7):
            cd8[pp_, s, kk // 2, kk % 2, pp_] = SCV * dww[s * P + pp_, kk]
    cd8 = np.clip(cd8, -240, 240).astype(E4).reshape(P, KC * 4 * 2 * P)

    m18 = np.full((P, 2 * 32), 1.0 / C, dtype=np.float32).astype(E4)
    ones4 = np.ones((1, P), dtype=np.float32)
    # dwb'' = dwb - b2 * sum_k w[c,k]   (conv input carries +b2)
    dwb_eff = dwb - b2p * dww.sum(axis=1) + bo * dww.sum(axis=1)
    # stored xres = x + b2p + Wo av = xres_true + b2  (xres_true has +bo)
    # conv(stored) = conv(xres_true) + b2 * sum_k w  ->  subtract b2*sum w
    dwb_eff = dwb - b2 * dww.sum(axis=1)
    dwb16 = (SY * dwb_eff).reshape(KC, P).T.copy()  # (P, KC)

    w18 = np.zeros((B_LOC * 8, P, KC, I), dtype=np.float32)  # placeholder big
    return dict(
        x=x, se_all=se_all, sh_all=sh_all, b2p=b2p, k8=k8, cjv=cjv, bvv=bvv,
        dlt=dlt, dblt=dblt, wo8=wo8, cd8=cd8, m18=m18, ones4=ones4,
        dwb16=dwb16, w1=w1, b1=b1, w2=w2, sh=sh_all,
    )


def kernel(**inputs):
    from concourse.bass_utils import run_bass_kernel_spmd

    pr = _prep(inputs)
    x, b2p = pr["x"], pr["b2p"]
    w1, b1, w2 = pr["w1"], pr["b1"], pr["w2"]
    se_all, sh_all = pr["se_all"], pr["sh_all"]

    xp = x + b2p[None, :, None]
    x8_all = np.clip(xp, -240, 240).astype(E4)

    w28 = np.clip(
        SW2 * w2.reshape(KI, P, C).transpose(1, 0, 2), -240, 240
    ).astype(E4).reshape(P, KI * C)

    in_maps = []
    for c in range(N_CORES):
        sl = slice(c * B_LOC, (c + 1) * B_LOC)
        w18_l = np.zeros((P, B_LOC, KC, I), dtype=np.float32)
        w1s8_l = np.zeros((1, B_LOC, 2, KI * P), dtype=np.float32)
        b1p_l = np.zeros((P, B_LOC, KI), dtype=np.float32)
        for bi, bglob in enumerate(range(c * B_LOC, (c + 1) * B_LOC)):
            se_b = se_all[bglob]
            sh_b = sh_all[bglob]
            w1p = se_b[:, None] * w1                 # (C, I)
            w18_l[:, bi] = (SW1 * w1p).reshape(KC, P, I).transpose(1, 0, 2)
            w1s8_l[0, bi, 0] = -SW1 * w1p.sum(axis=0)
            b1p_l[:, bi] = (b1 + sh_b @ w1).reshape(KI, P).T
        in_maps.append({
            "x": np.ascontiguousarray(xp[sl]),
            "x8": np.ascontiguousarray(x8_all[sl]),
            "k8": pr["k8"], "cj": pr["cjv"], "bv": pr["bvv"],
            "dlt": pr["dlt"], "dblt": pr["dblt"], "wo8": pr["wo8"],
            "cd8": pr["cd8"], "m18": pr["m18"], "ones4": pr["ones4"],
            "w18": np.clip(w18_l, -240, 240).astype(E4).reshape(P, B_LOC * KC * I),
            "w1s8": np.clip(w1s8_l, -240, 240).astype(E4).reshape(1, B_LOC * 2 * KI * P),
            "b1p": b1p_l.reshape(P, B_LOC * KI),
            "w28": w28,
            "dwb16": pr["dwb16"],
        })

    nc = _get_nc()
    res = run_bass_kernel_spmd(nc, in_maps, core_ids=list(range(N_CORES)))
    return np.concatenate([r["out"] for r in res.results], axis=0)


if __name__ == "__main__":
    nc = bass.Bass()
    build(nc)
    n = split_multiwaits(nc)
    print("built; multiwait splits:", n)
    from concourse.timeline_sim import TimelineSim
    nc2 = bass.Bass()
    build(nc2)
    split_multiwaits(nc2)
    print("TimelineSim:", TimelineSim(nc2).simulate())
